# revision 13
# baseline (speedup 1.0000x reference)
"""Trainium2 Bass kernel for nn_HDCNN (4-layer hyperbolic dilated CNN).

Data-parallel over 8 NeuronCores (4096 rows each). On-device layout is
feature-transposed: activations live as [feature, batch] tiles so the 64-tap
full convolution becomes banded 128x64 matmuls with static weights.

Math (validated against the reference): the Poincare projection always
triggers (row norms >> atanh(maxnorm)), so tanh cancels and each layer
reduces to   out = m * relu(conv_u + delta * y)   with per-sample scalars
from s = ||conv_u||^2 and d = <conv_u, y>. The per-sample scale m is
scale-invariant layer to layer, so only the last layer's m is applied.
"""
import os
import sys

for _p in ("/opt/trn_rl_repo", "/root/.axon_site/_ro/trn_rl_repo"):
    if _p not in sys.path and os.path.isdir(_p):
        sys.path.append(_p)

import numpy as np
import concourse.bacc as bacc
import concourse.mybir as mybir
import concourse.tile as tile
from concourse import bass_utils

F32 = mybir.dt.float32
F32R = mybir.dt.float32r
I32 = mybir.dt.int32
OP = mybir.AluOpType

NCORES = 8
BATCH = 32768
INSIZE = 1024
FLEN = 64
NUM_LAYERS = 4
ROWS_PER_CORE = BATCH // NCORES          # 4096
NB = 512                                  # batch columns per tile
NTILES = ROWS_PER_CORE // NB              # 8
MAXNORM = 1.0 - 4e-3
COEF_Y = 1.0 - MAXNORM * MAXNORM

LIN = [INSIZE + FLEN * i for i in range(NUM_LAYERS)]         # 1024 1088 1152 1216
LOUT = [l + FLEN for l in LIN]                                # 1088 1152 1216 1280
NIN = [(l + 127) // 128 for l in LIN]                         # 8 9 9 10
NOUT = [(l + 127) // 128 for l in LOUT]                       # 9 9 10 10

USE_FP32R = os.environ.get("HDCNN_FP32R", "1") == "1"


def _mm(ap):
    return ap.bitcast(F32R) if USE_FP32R else ap


def host_prep(w, b_list):
    """Replicated parameter layouts (repacking + tiny weight correlations)."""
    prep = {}
    WAB = np.zeros((NUM_LAYERS, 128, 64), np.float32)  # [A; B]
    W0A = np.zeros((NUM_LAYERS, 128, 64), np.float32)  # [0; A]
    WB0 = np.zeros((NUM_LAYERS, 128, 64), np.float32)  # [B; 0]
    for i in range(NUM_LAYERS):
        A = np.zeros((64, 64), np.float32)
        Bm = np.zeros((64, 64), np.float32)
        for a in range(64):
            for r in range(64):
                if 0 <= r - a < FLEN:
                    Bm[a, r] = w[i, r - a]
                if 0 <= 64 + r - a < FLEN:
                    A[a, r] = w[i, 64 + r - a]
        WAB[i, :64], WAB[i, 64:] = A, Bm
        W0A[i, 64:] = A
        WB0[i, :64] = Bm
    prep["wab"], prep["w0a"], prep["wb0"] = WAB, W0A, WB0

    nmax_in, nmax_out = max(NIN), max(NOUT)
    beta = np.zeros((NUM_LAYERS, 128, nmax_in), np.float32)
    ycol = np.zeros((NUM_LAYERS, 128, nmax_out), np.float32)
    y2cx = np.zeros((NUM_LAYERS, 128, 2), np.float32)
    for i in range(NUM_LAYERS):
        b64 = b_list[i].astype(np.float64)
        bt = np.correlate(b64, w[i].astype(np.float64), mode="valid")[: LIN[i]]
        bpad = np.zeros(NIN[i] * 128)
        bpad[: LIN[i]] = bt
        beta[i, :, : NIN[i]] = bpad.reshape(NIN[i], 128).T.astype(np.float32)
        ypad = np.zeros(NOUT[i] * 128)
        ypad[: LOUT[i]] = b64
        ycol[i, :, : NOUT[i]] = ypad.reshape(NOUT[i], 128).T.astype(np.float32)
        y2 = np.float32(np.sum(b_list[i].astype(np.float32) ** 2, dtype=np.float32))
        y2cx[i, :, 0] = np.float32(1.0) + np.float32(MAXNORM * MAXNORM) * y2
        y2cx[i, :, 1] = np.float32(1.0) + y2
    prep["beta"], prep["ycol"], prep["y2cx"] = beta, ycol, y2cx
    prep["ones"] = np.ones((128, 1), np.float32)
    prep["bcd"] = np.full((1, 128), COEF_Y / MAXNORM, np.float32)
    prep["bcm"] = np.full((1, 128), MAXNORM, np.float32)
    prep["id128"] = np.eye(128, dtype=np.float32)
    return prep


def build_program(ntiles=NTILES):
    nc = bacc.Bacc("TRN2", target_bir_lowering=False, debug=False)
    nmax_in, nmax_out = max(NIN), max(NOUT)
    ncols = ntiles * NB

    hkT = nc.dram_tensor("hkT", [INSIZE, ncols], F32, kind="ExternalInput")
    d_wab = nc.dram_tensor("wab", [NUM_LAYERS, 128, 64], F32, kind="ExternalInput")
    d_w0a = nc.dram_tensor("w0a", [NUM_LAYERS, 128, 64], F32, kind="ExternalInput")
    d_wb0 = nc.dram_tensor("wb0", [NUM_LAYERS, 128, 64], F32, kind="ExternalInput")
    d_beta = nc.dram_tensor("beta", [NUM_LAYERS, 128, nmax_in], F32, kind="ExternalInput")
    d_ycol = nc.dram_tensor("ycol", [NUM_LAYERS, 128, nmax_out], F32, kind="ExternalInput")
    d_y2cx = nc.dram_tensor("y2cx", [NUM_LAYERS, 128, 2], F32, kind="ExternalInput")
    d_ones = nc.dram_tensor("ones", [128, 1], F32, kind="ExternalInput")
    d_bcd = nc.dram_tensor("bcd", [1, 128], F32, kind="ExternalInput")
    d_bcm = nc.dram_tensor("bcm", [1, 128], F32, kind="ExternalInput")
    d_id = nc.dram_tensor("id128", [128, 128], F32, kind="ExternalInput")
    outT = nc.dram_tensor("outT", [LOUT[-1], ncols], F32, kind="ExternalOutput")

    hk_v = hkT.rearrange("(c p) n -> p c n", p=128)
    out_v = outT.rearrange("(c p) n -> p c n", p=128)

    with tile.TileContext(nc) as tc:
        with (
            tc.tile_pool(name="singles", bufs=1) as singles,
            tc.tile_pool(name="u0p", bufs=2) as u0p,
            tc.tile_pool(name="acts", bufs=1) as acts,
            tc.tile_pool(name="cvsb", bufs=2) as cvsbp,
            tc.tile_pool(name="sqp", bufs=3) as sqp,
            tc.tile_pool(name="outp", bufs=2) as outp,
            tc.tile_pool(name="smallp", bufs=2) as smallp,
            tc.tile_pool(name="cvps", bufs=3, space="PSUM") as cvps,
            tc.tile_pool(name="stps", bufs=1, space="PSUM") as stps,
            tc.tile_pool(name="tinyp", bufs=2, space="PSUM") as tinyps,
            tc.tile_pool(name="bcps", bufs=2, space="PSUM") as bcps,
        ):
            s_wab = singles.tile([128, NUM_LAYERS, 64], F32, tag="wab")
            s_w0a = singles.tile([128, NUM_LAYERS, 64], F32, tag="w0a")
            s_wb0 = singles.tile([128, NUM_LAYERS, 64], F32, tag="wb0")
            s_beta = singles.tile([128, NUM_LAYERS, nmax_in], F32, tag="beta")
            s_ycol = singles.tile([128, NUM_LAYERS, nmax_out], F32, tag="ycol")
            s_y2cx = singles.tile([128, NUM_LAYERS, 2], F32, tag="y2cx")
            s_ones = singles.tile([128, 1], F32, tag="ones")
            s_bcd = singles.tile([1, 128], F32, tag="bcd")
            s_bcm = singles.tile([1, 128], F32, tag="bcm")
            s_id = singles.tile([128, 128], F32, tag="id")
            nc.sync.dma_start(out=s_wab, in_=d_wab.rearrange("l p m -> p l m"))
            nc.sync.dma_start(out=s_w0a, in_=d_w0a.rearrange("l p m -> p l m"))
            nc.sync.dma_start(out=s_wb0, in_=d_wb0.rearrange("l p m -> p l m"))
            nc.sync.dma_start(out=s_beta, in_=d_beta.rearrange("l p m -> p l m"))
            nc.sync.dma_start(out=s_ycol, in_=d_ycol.rearrange("l p m -> p l m"))
            nc.sync.dma_start(out=s_y2cx, in_=d_y2cx.rearrange("l p m -> p l m"))
            nc.sync.dma_start(out=s_ones, in_=d_ones[:])
            nc.sync.dma_start(out=s_bcd, in_=d_bcd[:])
            nc.sync.dma_start(out=s_bcm, in_=d_bcm[:])
            nc.sync.dma_start(out=s_id, in_=d_id[:])

            for j in range(ntiles):
                ncol = slice(j * NB, (j + 1) * NB)
                u = u0p.tile([128, NIN[0], NB], F32, tag="u0")
                nc.sync.dma_start(out=u, in_=hk_v[:, :, ncol])

                for li in range(NUM_LAYERS):
                    lin, lout = LIN[li], LOUT[li]
                    nin, nout = NIN[li], NOUT[li]
                    nvb = lin // 64
                    tout = lout // 64
                    last = li == NUM_LAYERS - 1
                    wab_l = s_wab[:, li, :]
                    w0a_l = s_w0a[:, li, :]
                    wb0_l = s_wb0[:, li, :]

                    stats = stps.tile([128, NB], F32, tag="st")
                    for ch in range(nin):
                        k = 128 if (ch + 1) * 128 <= lin else 64
                        nc.tensor.matmul(
                            stats[32:33, :],
                            _mm(s_beta[:k, li, ch: ch + 1]),
                            _mm(u[:k, ch, :]),
                            start=(ch == 0), stop=(ch == nin - 1),
                            tile_position=(0, 32),
                        )

                    cvsb = cvsbp.tile([128, nout, NB], F32, tag="cvsb")
                    sq_tiles = []
                    for c in range(nout):
                        outv = 128 if (c + 1) * 128 <= lout else 64
                        pcv = cvps.tile([128, NB], F32, tag="cv")
                        for hi in (0, 1):
                            t = 2 * c + hi
                            if t >= tout:
                                continue
                            contribs = []
                            if 0 <= t - 1 < nvb:
                                contribs.append((t - 1, "A"))
                            if t < nvb:
                                contribs.append((t, "B"))
                            out_ap = pcv[64 * hi: 64 * hi + 64, :]
                            if hi == 1 and len(contribs) == 2:
                                ch = (t - 1) // 2
                                nc.tensor.matmul(
                                    out_ap, _mm(wab_l), _mm(u[:, ch, :]),
                                    start=True, stop=True, tile_position=(0, 64),
                                )
                                continue
                            for ci, (vb, ab) in enumerate(contribs):
                                ch, h = vb // 2, vb % 2
                                full = (ch + 1) * 128 <= lin
                                if full:
                                    lhs = w0a_l if ab == "A" and h == 1 else (
                                        wb0_l if ab == "B" and h == 0 else None)
                                    if lhs is None:  # A at rows 0-63 / B at rows 64-127
                                        lhs = wab_l if ab == "A" else None
                                        assert ab == "A" and h == 0
                                        # half-row K=64 (mixed-K accum is ok)
                                        nc.tensor.matmul(
                                            out_ap, _mm(wab_l[0:64, :]),
                                            _mm(u[0:64, ch, :]),
                                            start=(ci == 0),
                                            stop=(ci == len(contribs) - 1),
                                            tile_position=(0, 64 * hi),
                                        )
                                        continue
                                    nc.tensor.matmul(
                                        out_ap, _mm(lhs), _mm(u[:, ch, :]),
                                        start=(ci == 0),
                                        stop=(ci == len(contribs) - 1),
                                        tile_position=(0, 64 * hi),
                                    )
                                else:
                                    # half chunk: valid rows 0-63 only
                                    assert h == 0
                                    lhs = wab_l[0:64, :] if ab == "A" else wb0_l[0:64, :]
                                    nc.tensor.matmul(
                                        out_ap, _mm(lhs), _mm(u[0:64, ch, :]),
                                        start=(ci == 0),
                                        stop=(ci == len(contribs) - 1),
                                        tile_position=(0, 64 * hi),
                                    )
                        nc.scalar.copy(cvsb[:outv, c, :], pcv[:outv, :])
                        sq = sqp.tile([128, NB], F32, tag="sq")
                        nc.scalar.square(sq[:outv, :], pcv[:outv, :])
                        sq_tiles.append((sq, outv))
                    for c, (sq, outv) in enumerate(sq_tiles):
                        nc.tensor.matmul(
                            stats[0:1, :], _mm(s_ones[:outv, :]), _mm(sq[:outv, :]),
                            start=(c == 0), stop=(c == nout - 1),
                            tile_position=(0, 0),
                        )

                    # ---- per-sample scalars, in [128, 4] layout ----
                    sd_sb = smallp.tile([1, 2 * NB], F32, tag="sdsb")
                    nc.vector.tensor_copy(sd_sb[0:1, 0:NB], stats[0:1, :])
                    nc.vector.tensor_copy(sd_sb[0:1, NB: 2 * NB], stats[32:33, :])
                    tp = tinyps.tile([128, 128], F32, tag="tiny")
                    for p in range(8):
                        nc.tensor.transpose(
                            tp[:, p: p + 1],
                            sd_sb[0:1, 128 * p: 128 * p + 128],
                            s_id[:1, :1],
                        )
                    sc = smallp.tile([128, 40], F32, tag="sc")
                    sci = sc.bitcast(I32)
                    nc.vector.tensor_copy(sc[:, 0:8], tp[:, 0:8])
                    S, D = sc[:, 0:4], sc[:, 4:8]
                    Si = sci[:, 0:4]

                    def col4(k):
                        return sc[:, 8 + 4 * k: 12 + 4 * k]

                    r, t1, t2, sqs, t0, den, cx, P = (col4(k) for k in range(8))
                    ri32 = sci[:, 8:12]
                    # rsqrt(s): quake seed + 3 Newton steps
                    nc.vector.tensor_scalar(
                        ri32, Si, 1, None, OP.logical_shift_right)
                    nc.vector.tensor_scalar(
                        ri32, ri32, 0x5F3759DF, -1, OP.subtract, OP.mult)
                    for _ in range(3):
                        nc.vector.tensor_tensor(t1, S, r, OP.mult)
                        nc.vector.tensor_tensor(t2, t1, r, OP.mult)
                        nc.vector.tensor_scalar(t2, t2, -0.5, 1.5, OP.mult, OP.add)
                        nc.vector.tensor_tensor(r, r, t2, OP.mult)
                    nc.vector.tensor_tensor(sqs, S, r, OP.mult)         # sqrt(s)
                    nc.vector.tensor_tensor(t0, D, r, OP.mult)          # d / sqrt(s)
                    nc.vector.tensor_scalar(
                        den, t0, 2.0 * MAXNORM, s_y2cx[:, li, 0:1], OP.mult, OP.add)
                    nc.vector.tensor_scalar(
                        cx, t0, 2.0 * MAXNORM, s_y2cx[:, li, 1:2], OP.mult, OP.add)
                    nc.vector.tensor_tensor(t1, cx, den, OP.mult)
                    nc.vector.reciprocal(P, t1)                         # 1/(cx*den)
                    dm = smallp.tile([128, 8], F32, tag="dm")
                    nc.vector.tensor_tensor(t2, sqs, den, OP.mult)
                    nc.vector.tensor_tensor(dm[:, 0:4], t2, P, OP.mult)  # sqrt(s)/cx
                    nrow = 4
                    if last:
                        nc.vector.tensor_tensor(t2, cx, cx, OP.mult)
                        nc.vector.tensor_tensor(t1, t2, r, OP.mult)
                        nc.vector.tensor_tensor(dm[:, 4:8], t1, P, OP.mult)  # cx*r/den
                        nrow = 8
                    def bc_build(col0, lhs_const, pool_tag):
                        btp = tinyps.tile([1, 512], F32, tag="tiny")
                        for k in range(4):
                            nc.tensor.transpose(
                                btp[0:1, 128 * k: 128 * (k + 1)],
                                dm[:, col0 + k: col0 + k + 1], s_id)
                        rws = smallp.tile([1, 512], F32, tag="rows")
                        nc.scalar.copy(rws[0:1, :], btp[0:1, :])
                        bc = bcps.tile([128, NB], F32, tag="bc")
                        for p in range(4):
                            nc.tensor.matmul(
                                bc[:, 128 * p: 128 * (p + 1)], lhs_const,
                                rws[0:1, 128 * p: 128 * (p + 1)],
                                start=True, stop=True, tile_position=(0, 0),
                            )
                        return bc

                    dbc = bc_build(0, s_bcd, "bc")
                    if last:
                        mbc = bc_build(4, s_bcm, "bc")

                    # ---- z: q = conv + delta*y (in place on cvsb) ----
                    for c in range(nout):
                        outv = 128 if (c + 1) * 128 <= lout else 64
                        nc.vector.scalar_tensor_tensor(
                            cvsb[:outv, c, :], dbc[:outv, :],
                            s_ycol[:outv, li, c: c + 1], cvsb[:outv, c, :],
                            OP.mult, OP.add,
                        )

                    if not last:
                        un = acts.tile([128, NOUT[li], NB], F32, tag=f"u{li + 1}")
                        for c in range(nout):
                            outv = 128 if (c + 1) * 128 <= lout else 64
                            nc.gpsimd.tensor_scalar_max(
                                un[:outv, c, :], cvsb[:outv, c, :], 0.0)
                        u = un
                    else:
                        ot = outp.tile([128, nout, NB], F32, tag="out")
                        for c in range(nout):
                            nc.vector.tensor_tensor(
                                ot[:, c, :], cvsb[:, c, :], mbc, OP.mult)
                        flat = ot.rearrange("p c n -> p (c n)")
                        nc.vector.tensor_scalar_max(flat, flat, 0.0)
                        nc.sync.dma_start(out=out_v[:, :, ncol], in_=ot)

    nc.compile()
    return nc


_NC_CACHE = {}


def _get_program(ntiles=NTILES):
    if ntiles not in _NC_CACHE:
        _NC_CACHE[ntiles] = build_program(ntiles)
    return _NC_CACHE[ntiles]


def kernel(**inputs):
    hk = np.asarray(inputs["hk"], dtype=np.float32)
    w = np.asarray(inputs["w"], dtype=np.float32)
    b_list = [np.asarray(inputs[f"b{i}"], dtype=np.float32) for i in range(NUM_LAYERS)]

    prep = host_prep(w, b_list)
    nc = _get_program()

    in_maps = []
    for c in range(NCORES):
        rows = slice(c * ROWS_PER_CORE, (c + 1) * ROWS_PER_CORE)
        m = dict(prep)
        m["hkT"] = np.ascontiguousarray(hk[rows].T)
        in_maps.append(m)

    res = bass_utils.run_bass_kernel_spmd(nc, in_maps, list(range(NCORES)))
    outs = [np.asarray(res.results[c]["outT"]).T for c in range(NCORES)]
    return np.ascontiguousarray(np.concatenate(outs, axis=0))


# revision 19
# speedup vs baseline: 3567.6045x; 3567.6045x over previous
"""Trainium2 Bass kernel for nn_HDCNN (4-layer hyperbolic dilated CNN).

Data-parallel over 8 NeuronCores (4096 rows each). On-device layout is
feature-transposed: activations live as [feature, batch] tiles so the 64-tap
full convolution becomes banded 128x64 matmuls with static weights.

Math (validated against the reference): the Poincare projection always
triggers (row norms >> atanh(maxnorm)), so tanh cancels and each layer
reduces to   out = m * relu(conv_u + delta * y)   with per-sample scalars
from s = ||conv_u||^2 and d = <conv_u, y>. The per-sample scale m is
scale-invariant layer to layer, so only the last layer's m is applied.
"""
import os
import sys

for _p in ("/opt/trn_rl_repo", "/root/.axon_site/_ro/trn_rl_repo"):
    if _p not in sys.path and os.path.isdir(_p):
        sys.path.append(_p)

import numpy as np
import concourse.bacc as bacc
import concourse.mybir as mybir
import concourse.tile as tile
from concourse import bass_utils

F32 = mybir.dt.float32
F32R = mybir.dt.float32r
I32 = mybir.dt.int32
OP = mybir.AluOpType

NCORES = 8
BATCH = 32768
INSIZE = 1024
FLEN = 64
NUM_LAYERS = 4
ROWS_PER_CORE = BATCH // NCORES          # 4096
NB = 512                                  # batch columns per tile
NTILES = ROWS_PER_CORE // NB              # 8
MAXNORM = 1.0 - 4e-3
COEF_Y = 1.0 - MAXNORM * MAXNORM

LIN = [INSIZE + FLEN * i for i in range(NUM_LAYERS)]         # 1024 1088 1152 1216
LOUT = [l + FLEN for l in LIN]                                # 1088 1152 1216 1280
NIN = [(l + 127) // 128 for l in LIN]                         # 8 9 9 10
NOUT = [(l + 127) // 128 for l in LOUT]                       # 9 9 10 10

USE_FP32R = os.environ.get("HDCNN_FP32R", "0") == "1"
MDT = F32R if USE_FP32R else F32


def _mm(ap):
    return ap.bitcast(F32) if ap.dtype == F32R else ap


def host_prep(w, b_list):
    """Replicated parameter layouts (repacking + tiny weight correlations)."""
    prep = {}
    WAB = np.zeros((NUM_LAYERS, 128, 64), np.float32)  # [A; B]
    W0A = np.zeros((NUM_LAYERS, 128, 64), np.float32)  # [0; A]
    WB0 = np.zeros((NUM_LAYERS, 128, 64), np.float32)  # [B; 0]
    for i in range(NUM_LAYERS):
        A = np.zeros((64, 64), np.float32)
        Bm = np.zeros((64, 64), np.float32)
        for a in range(64):
            for r in range(64):
                if 0 <= r - a < FLEN:
                    Bm[a, r] = w[i, r - a]
                if 0 <= 64 + r - a < FLEN:
                    A[a, r] = w[i, 64 + r - a]
        WAB[i, :64], WAB[i, 64:] = A, Bm
        W0A[i, 64:] = A
        WB0[i, :64] = Bm
    prep["wab"], prep["w0a"], prep["wb0"] = WAB, W0A, WB0

    nmax_in, nmax_out = max(NIN), max(NOUT)
    beta = np.zeros((NUM_LAYERS, 128, nmax_in), np.float32)
    ycol = np.zeros((NUM_LAYERS, 128, nmax_out), np.float32)
    y2cx = np.zeros((NUM_LAYERS, 128, 2), np.float32)
    for i in range(NUM_LAYERS):
        b64 = b_list[i].astype(np.float64)
        bt = np.correlate(b64, w[i].astype(np.float64), mode="valid")[: LIN[i]]
        bpad = np.zeros(NIN[i] * 128)
        bpad[: LIN[i]] = bt
        beta[i, :, : NIN[i]] = bpad.reshape(NIN[i], 128).T.astype(np.float32)
        ypad = np.zeros(NOUT[i] * 128)
        ypad[: LOUT[i]] = b64
        ycol[i, :, : NOUT[i]] = ypad.reshape(NOUT[i], 128).T.astype(np.float32)
        y2 = np.float32(np.sum(b_list[i].astype(np.float32) ** 2, dtype=np.float32))
        y2cx[i, :, 0] = np.float32(1.0) + np.float32(MAXNORM * MAXNORM) * y2
        y2cx[i, :, 1] = np.float32(1.0) + y2
    prep["beta"], prep["ycol"], prep["y2cx"] = beta, ycol, y2cx
    prep["ones"] = np.ones((128, 1), np.float32)
    prep["bcd"] = np.full((1, 128), COEF_Y / MAXNORM, np.float32)
    prep["bcm"] = np.full((1, 128), MAXNORM, np.float32)
    prep["id128"] = np.eye(128, dtype=np.float32)
    return prep


def build_program(ntiles=NTILES, reps=1):
    nc = bacc.Bacc("TRN2", target_bir_lowering=False, debug=False)
    nmax_in, nmax_out = max(NIN), max(NOUT)
    ncols = ntiles * NB

    hkT = nc.dram_tensor("hkT", [INSIZE, ncols], MDT, kind="ExternalInput")
    d_wab = nc.dram_tensor("wab", [NUM_LAYERS, 128, 64], F32, kind="ExternalInput")
    d_w0a = nc.dram_tensor("w0a", [NUM_LAYERS, 128, 64], F32, kind="ExternalInput")
    d_wb0 = nc.dram_tensor("wb0", [NUM_LAYERS, 128, 64], F32, kind="ExternalInput")
    d_beta = nc.dram_tensor("beta", [NUM_LAYERS, 128, nmax_in], MDT, kind="ExternalInput")
    d_ycol = nc.dram_tensor("ycol", [NUM_LAYERS, 128, nmax_out], F32, kind="ExternalInput")
    d_y2cx = nc.dram_tensor("y2cx", [NUM_LAYERS, 128, 2], F32, kind="ExternalInput")
    d_ones = nc.dram_tensor("ones", [128, 1], MDT, kind="ExternalInput")
    d_bcd = nc.dram_tensor("bcd", [1, 128], F32, kind="ExternalInput")
    d_bcm = nc.dram_tensor("bcm", [1, 128], F32, kind="ExternalInput")
    d_id = nc.dram_tensor("id128", [128, 128], F32, kind="ExternalInput")
    outT = nc.dram_tensor("outT", [LOUT[-1], ncols], F32, kind="ExternalOutput")

    hk_v = hkT.rearrange("(c p) n -> p c n", p=128)
    out_v = outT.rearrange("(c p) n -> p c n", p=128)

    with tile.TileContext(nc) as tc:
        with (
            tc.tile_pool(name="singles", bufs=1) as singles,
            tc.tile_pool(name="u0p", bufs=2) as u0p,
            tc.tile_pool(name="acts", bufs=1) as acts,
            tc.tile_pool(name="cvsb", bufs=2) as cvsbp,
            tc.tile_pool(name="sqp", bufs=3) as sqp,
            tc.tile_pool(name="outp", bufs=2) as outp,
            tc.tile_pool(name="smallp", bufs=2) as smallp,
            tc.tile_pool(name="cvps", bufs=2, space="PSUM") as cvps,
            tc.tile_pool(name="stps", bufs=1, space="PSUM") as stps,
            tc.tile_pool(name="tinyp", bufs=1, space="PSUM") as tinyps,
            tc.tile_pool(name="bcps", bufs=2, space="PSUM") as bcps,
        ):
            s_wab = singles.tile([128, NUM_LAYERS, 64], F32, tag="wab")
            s_w0a = singles.tile([128, NUM_LAYERS, 64], F32, tag="w0a")
            s_wb0 = singles.tile([128, NUM_LAYERS, 64], F32, tag="wb0")
            s_beta = singles.tile([128, NUM_LAYERS, nmax_in], MDT, tag="beta")
            s_ycol = singles.tile([128, NUM_LAYERS, nmax_out], F32, tag="ycol")
            s_y2cx = singles.tile([128, NUM_LAYERS, 2], F32, tag="y2cx")
            s_ones = singles.tile([128, 1], MDT, tag="ones")
            s_bcd = singles.tile([1, 128], F32, tag="bcd")
            s_bcm = singles.tile([1, 128], F32, tag="bcm")
            s_id = singles.tile([128, 128], F32, tag="id")
            nc.sync.dma_start(out=s_wab, in_=d_wab.rearrange("l p m -> p l m"))
            nc.sync.dma_start(out=s_w0a, in_=d_w0a.rearrange("l p m -> p l m"))
            nc.sync.dma_start(out=s_wb0, in_=d_wb0.rearrange("l p m -> p l m"))
            nc.sync.dma_start(out=s_beta, in_=d_beta.rearrange("l p m -> p l m"))
            nc.sync.dma_start(out=s_ycol, in_=d_ycol.rearrange("l p m -> p l m"))
            nc.sync.dma_start(out=s_y2cx, in_=d_y2cx.rearrange("l p m -> p l m"))
            nc.sync.dma_start(out=s_ones, in_=d_ones[:])
            nc.sync.dma_start(out=s_bcd, in_=d_bcd[:])
            nc.sync.dma_start(out=s_bcm, in_=d_bcm[:])
            nc.sync.dma_start(out=s_id, in_=d_id[:])

            for j in range(ntiles * reps):
                j = j % ntiles
                ncol = slice(j * NB, (j + 1) * NB)
                u = u0p.tile([128, NIN[0], NB], MDT, tag="u0")
                nc.sync.dma_start(out=u, in_=hk_v[:, :, ncol])

                for li in range(NUM_LAYERS):
                    lin, lout = LIN[li], LOUT[li]
                    nin, nout = NIN[li], NOUT[li]
                    nvb = lin // 64
                    tout = lout // 64
                    last = li == NUM_LAYERS - 1
                    wab_l = s_wab[:, li, :]
                    w0a_l = s_w0a[:, li, :]
                    wb0_l = s_wb0[:, li, :]

                    stats_s = stps.tile([1, NB], F32, tag="sts")
                    stats_d = stps.tile([1, NB], F32, tag="std")
                    for ch in range(nin):
                        k = 128 if (ch + 1) * 128 <= lin else 64
                        nc.tensor.matmul(
                            stats_d[0:1, :],
                            s_beta[:k, li, ch: ch + 1],
                            u[:k, ch, :],
                            start=(ch == 0), stop=(ch == nin - 1),
                            tile_position=(0, 0),
                        )

                    cvsb = cvsbp.tile([128, nout, NB], F32, tag="cvsb")
                    sq_tiles = []
                    for c in range(nout):
                        outv = 128 if (c + 1) * 128 <= lout else 64
                        pcv = cvps.tile([128, NB], F32, tag="cv")
                        for hi in (0, 1):
                            t = 2 * c + hi
                            if t >= tout:
                                continue
                            contribs = []
                            if 0 <= t - 1 < nvb:
                                contribs.append((t - 1, "A"))
                            if t < nvb:
                                contribs.append((t, "B"))
                            out_ap = pcv[64 * hi: 64 * hi + 64, :]
                            if hi == 1 and len(contribs) == 2:
                                ch = (t - 1) // 2
                                nc.tensor.matmul(
                                    out_ap, _mm(wab_l), _mm(u[:, ch, :]),
                                    start=True, stop=True, tile_position=(0, 64),
                                )
                                continue
                            for ci, (vb, ab) in enumerate(contribs):
                                ch, h = vb // 2, vb % 2
                                full = (ch + 1) * 128 <= lin
                                if full:
                                    lhs = w0a_l if ab == "A" and h == 1 else (
                                        wb0_l if ab == "B" and h == 0 else None)
                                    if lhs is None:  # A at rows 0-63 / B at rows 64-127
                                        lhs = wab_l if ab == "A" else None
                                        assert ab == "A" and h == 0
                                        # half-row K=64 (mixed-K accum is ok)
                                        nc.tensor.matmul(
                                            out_ap, _mm(wab_l[0:64, :]),
                                            _mm(u[0:64, ch, :]),
                                            start=(ci == 0),
                                            stop=(ci == len(contribs) - 1),
                                            tile_position=(0, 64 * hi),
                                        )
                                        continue
                                    nc.tensor.matmul(
                                        out_ap, _mm(lhs), _mm(u[:, ch, :]),
                                        start=(ci == 0),
                                        stop=(ci == len(contribs) - 1),
                                        tile_position=(0, 64 * hi),
                                    )
                                else:
                                    # half chunk: valid rows 0-63 only
                                    assert h == 0
                                    lhs = wab_l[0:64, :] if ab == "A" else wb0_l[0:64, :]
                                    nc.tensor.matmul(
                                        out_ap, _mm(lhs), _mm(u[0:64, ch, :]),
                                        start=(ci == 0),
                                        stop=(ci == len(contribs) - 1),
                                        tile_position=(0, 64 * hi),
                                    )
                        nc.scalar.copy(cvsb[:outv, c, :], pcv[:outv, :])
                        sq = sqp.tile([128, NB], MDT, tag="sq")
                        nc.scalar.square(sq[:outv, :], pcv[:outv, :])
                        sq_tiles.append((sq, outv))
                    for c, (sq, outv) in enumerate(sq_tiles):
                        nc.tensor.matmul(
                            stats_s[0:1, :], s_ones[:outv, :], sq[:outv, :],
                            start=(c == 0), stop=(c == nout - 1),
                            tile_position=(0, 0),
                        )

                    # ---- per-sample scalars, in [128, 4] layout ----
                    sd_sb = smallp.tile([1, 2 * NB], F32, tag="sdsb")
                    nc.vector.tensor_copy(sd_sb[0:1, 0:NB], stats_s[0:1, :])
                    nc.vector.tensor_copy(sd_sb[0:1, NB: 2 * NB], stats_d[0:1, :])
                    tp = tinyps.tile([128, 128], F32, tag="tiny")
                    for p in range(8):
                        nc.tensor.transpose(
                            tp[:, p: p + 1],
                            sd_sb[0:1, 128 * p: 128 * p + 128],
                            s_id[:1, :1],
                        )
                    sc = smallp.tile([128, 40], F32, tag="sc")
                    sci = sc.bitcast(I32)
                    nc.vector.tensor_copy(sc[:, 0:8], tp[:, 0:8])
                    S, D = sc[:, 0:4], sc[:, 4:8]
                    Si = sci[:, 0:4]

                    def col4(k):
                        return sc[:, 8 + 4 * k: 12 + 4 * k]

                    r, t1, t2, sqs, t0, den, cx, P = (col4(k) for k in range(8))
                    ri32 = sci[:, 8:12]
                    # rsqrt(s): quake seed + 3 Newton steps
                    nc.vector.tensor_scalar(
                        ri32, Si, 1, None, OP.logical_shift_right)
                    nc.vector.tensor_scalar(
                        ri32, ri32, 0x5F3759DF, -1, OP.subtract, OP.mult)
                    for _ in range(3):
                        nc.vector.tensor_tensor(t1, S, r, OP.mult)
                        nc.vector.tensor_tensor(t2, t1, r, OP.mult)
                        nc.vector.tensor_scalar(t2, t2, -0.5, 1.5, OP.mult, OP.add)
                        nc.vector.tensor_tensor(r, r, t2, OP.mult)
                    nc.vector.tensor_tensor(sqs, S, r, OP.mult)         # sqrt(s)
                    nc.vector.tensor_tensor(t0, D, r, OP.mult)          # d / sqrt(s)
                    nc.vector.tensor_scalar(
                        den, t0, 2.0 * MAXNORM, s_y2cx[:, li, 0:1], OP.mult, OP.add)
                    nc.vector.tensor_scalar(
                        cx, t0, 2.0 * MAXNORM, s_y2cx[:, li, 1:2], OP.mult, OP.add)
                    nc.vector.tensor_tensor(t1, cx, den, OP.mult)
                    nc.vector.reciprocal(P, t1)                         # 1/(cx*den)
                    dm = smallp.tile([128, 8], F32, tag="dm")
                    nc.vector.tensor_tensor(t2, sqs, den, OP.mult)
                    nc.vector.tensor_tensor(dm[:, 0:4], t2, P, OP.mult)  # sqrt(s)/cx
                    nrow = 4
                    if last:
                        nc.vector.tensor_tensor(t2, cx, cx, OP.mult)
                        nc.vector.tensor_tensor(t1, t2, r, OP.mult)
                        nc.vector.tensor_tensor(dm[:, 4:8], t1, P, OP.mult)  # cx*r/den
                        nrow = 8
                    def bc_build(col0, lhs_const, pool_tag):
                        btp = tinyps.tile([1, 512], F32, tag="tiny")
                        for k in range(4):
                            nc.tensor.transpose(
                                btp[0:1, 128 * k: 128 * (k + 1)],
                                dm[:, col0 + k: col0 + k + 1], s_id)
                        rws = smallp.tile([1, 512], F32, tag="rows")
                        nc.scalar.copy(rws[0:1, :], btp[0:1, :])
                        bc = bcps.tile([128, NB], F32, tag="bc")
                        for p in range(4):
                            nc.tensor.matmul(
                                bc[:, 128 * p: 128 * (p + 1)], lhs_const,
                                rws[0:1, 128 * p: 128 * (p + 1)],
                                start=True, stop=True, tile_position=(0, 0),
                            )
                        return bc

                    dbc = bc_build(0, s_bcd, "bc")
                    if last:
                        mbc = bc_build(4, s_bcm, "bc")

                    # ---- z: q = conv + delta*y (in place on cvsb) ----
                    for c in range(nout):
                        outv = 128 if (c + 1) * 128 <= lout else 64
                        nc.vector.scalar_tensor_tensor(
                            cvsb[:outv, c, :], dbc[:outv, :],
                            s_ycol[:outv, li, c: c + 1], cvsb[:outv, c, :],
                            OP.mult, OP.add,
                        )

                    if not last:
                        un = acts.tile([128, NOUT[li], NB], MDT, tag=f"u{li + 1}")
                        for c in range(nout):
                            outv = 128 if (c + 1) * 128 <= lout else 64
                            nc.gpsimd.tensor_scalar_max(
                                un[:outv, c, :], cvsb[:outv, c, :], 0.0)
                        u = un
                    else:
                        ot = outp.tile([128, nout, NB], F32, tag="out")
                        for c in range(nout):
                            nc.vector.tensor_tensor(
                                ot[:, c, :], cvsb[:, c, :], mbc, OP.mult)
                        flat = ot.rearrange("p c n -> p (c n)")
                        nc.vector.tensor_scalar_max(flat, flat, 0.0)
                        nc.sync.dma_start(out=out_v[:, :, ncol], in_=ot)

    nc.compile()
    return nc


_NC_CACHE = {}


def _get_program(ntiles=NTILES):
    if ntiles not in _NC_CACHE:
        _NC_CACHE[ntiles] = build_program(ntiles)
    return _NC_CACHE[ntiles]


def kernel(**inputs):
    hk = np.asarray(inputs["hk"], dtype=np.float32)
    w = np.asarray(inputs["w"], dtype=np.float32)
    b_list = [np.asarray(inputs[f"b{i}"], dtype=np.float32) for i in range(NUM_LAYERS)]

    prep = host_prep(w, b_list)
    nc = _get_program()

    in_maps = []
    for c in range(NCORES):
        rows = slice(c * ROWS_PER_CORE, (c + 1) * ROWS_PER_CORE)
        m = dict(prep)
        m["hkT"] = np.ascontiguousarray(hk[rows].T)
        in_maps.append(m)

    res = bass_utils.run_bass_kernel_spmd(nc, in_maps, list(range(NCORES)))
    outs = [np.asarray(res.results[c]["outT"]).T for c in range(NCORES)]
    return np.ascontiguousarray(np.concatenate(outs, axis=0))
